# revision 4
# baseline (speedup 1.0000x reference)
"""GRU-style GNN message-passing kernel for Trainium2 (8 NeuronCores, SPMD).

Reference computation (per node b, features 256, 8 neighbors):
    xr = x @ Wir.T + bir
    hr_n = hs_n @ Whr.T + bhr
    r_n = sigmoid(xr + hr_n)
    z = sigmoid(x @ Wiz.T + biz + h_sum @ Whz.T + bhz)
    s = sum_n r_n * hs_n
    n = tanh(x @ Win.T + bin + s @ Whn.T + bhn)
    out = (1 - z) * n + z * h_sum

Strategy: data-parallel over B=32768 across 8 cores (4096 rows each),
8 batch-chunks of 512 per core, feature-major on-chip layout
([256 features = 2 partition chunks of 128, batch free dim]).

v2 vs the first working version: engineered for PE density (the wall is
~116us of bf16 matmul columns; any PE idle gap also re-engages the HAM
clock throttle), host-side pre-chunked HBM layouts so every DMA is a
plain 2D copy with 2-4KB contiguous runs, h_sum shipped as both f32 and
bf16 (no on-chip cast), wider ACT instructions, DVE ops that all hit the
2x bf16 mode, and the n-gate of chunk c software-pipelined into the
middle of chunk c+1 so PE never waits on the DVE product tree.

Engine placement per chunk of 512 nodes:
  - PE: 68 matmuls @512 cols (z 8, xr 4, r 48 incl. per-neighbor
    identity adds of xr, n 8), PSUM-accumulated.
  - ACT: sigmoid/tanh/bias (z 2, xr 2, r 8 wide @1024, n 2).
  - DVE: r*hs pair products + neighbor add tree in bf16 2x mode; final
    combine for the last chunk (tail latency).
  - GPSIMD: final combine out = n + z*(h - n) for chunks 0..6, fp32.
"""

import sys
import numpy as np
from contextlib import ExitStack

sys.path.insert(0, "/opt/trn_rl_repo")

import ml_dtypes
import concourse.bacc as bacc
import concourse.tile as tile
from concourse import mybir
from concourse.bass_utils import run_bass_kernel_spmd

F32 = mybir.dt.float32
BF16 = mybir.dt.bfloat16
BF_NP = ml_dtypes.bfloat16

N_NEIGH, B, IN, H = 8, 32768, 256, 256
M = 8                    # cores
BL = B // M              # rows per core (4096)
NCH = 8                  # batch chunks per core
CW = BL // NCH           # chunk width (512)
NPAIR = N_NEIGH // 2     # neighbor pairs (4)

_cached = None  # compiled program, reused across kernel() calls

SIG = mybir.ActivationFunctionType.Sigmoid
TANH = mybir.ActivationFunctionType.Tanh


def _build():
    nc = bacc.Bacc("TRN2", target_bir_lowering=False, debug=False, num_devices=M)

    xL = nc.dram_tensor("xL", [NCH, 128, 1024], BF16, kind="ExternalInput").ap()
    hfL = nc.dram_tensor("hfL", [NCH, 128, 1024], F32, kind="ExternalInput").ap()
    hbL = nc.dram_tensor("hbL", [NCH, 128, 1024], BF16, kind="ExternalInput").ap()
    hsL = nc.dram_tensor("hsL", [NCH, NPAIR, 128, 2048], BF16,
                         kind="ExternalInput").ap()
    wAP = {}
    for w in ("wir", "whr", "wiz", "whz", "win", "whn"):
        wAP[w] = nc.dram_tensor(w, [256, 256], BF16, kind="ExternalInput").ap()
    ident = nc.dram_tensor("ident", [128, 128], BF16, kind="ExternalInput").ap()
    # bias pack: col f*3+j holds feature-chunk f of (b_r, b_z, b_n)[j]
    biasp = nc.dram_tensor("biasp", [128, 6], F32, kind="ExternalInput").ap()
    outL = nc.dram_tensor("outL", [NCH, 128, 1024], F32, kind="ExternalOutput").ap()

    with tile.TileContext(nc) as tc, ExitStack() as ctx:
        cpool = ctx.enter_context(tc.tile_pool(name="const", bufs=1))
        x_pool = ctx.enter_context(tc.tile_pool(name="x", bufs=2))
        hf_pool = ctx.enter_context(tc.tile_pool(name="hf", bufs=2))
        hb_pool = ctx.enter_context(tc.tile_pool(name="hb", bufs=2))
        hs_pool = ctx.enter_context(tc.tile_pool(name="hs", bufs=2))
        xr_pool = ctx.enter_context(tc.tile_pool(name="xr", bufs=2))
        z_pool = ctx.enter_context(tc.tile_pool(name="z", bufs=2))
        rc_pool = ctx.enter_context(tc.tile_pool(name="rc", bufs=2))
        s_pool = ctx.enter_context(tc.tile_pool(name="s", bufs=2))
        n_pool = ctx.enter_context(tc.tile_pool(name="n", bufs=2))
        d_pool = ctx.enter_context(tc.tile_pool(name="d", bufs=2))
        o_pool = ctx.enter_context(tc.tile_pool(name="o", bufs=2))
        pg_pool = ctx.enter_context(tc.tile_pool(name="pg", bufs=2, space="PSUM"))
        pr_pool = ctx.enter_context(tc.tile_pool(name="pr", bufs=2, space="PSUM"))

        # --- constants ---
        wt = {}
        for w in ("wir", "whr", "wiz", "whz", "win", "whn"):
            wt[w] = []
            for k in range(2):
                t = cpool.tile([128, 256], BF16, tag=f"{w}{k}", name=f"{w}{k}")
                nc.sync.dma_start(out=t[:, :], in_=wAP[w][k * 128:(k + 1) * 128, :])
                wt[w].append(t)
        id_t = cpool.tile([128, 128], BF16, tag="ident", name="id_t")
        nc.sync.dma_start(out=id_t[:, :], in_=ident[:, :])
        bias_t = cpool.tile([128, 6], F32, tag="biasp", name="bias_t")
        nc.sync.dma_start(out=bias_t[:, :], in_=biasp[:, :])

        def fcols(t, f):
            return t[:, f * 128:(f + 1) * 128]

        state = {}  # chunk -> tiles needed by the deferred n-gate/combine

        def emit_tail(cc, on_dve):
            st = state.pop(cc)
            pn = pg_pool.tile([128, 1024], F32, tag="pg", name=f"pn_{cc}")
            for fi in range(2):
                o = pn[:, fi * 512:(fi + 1) * 512]
                nc.tensor.matmul(o, fcols(wt["win"][0], fi), st["x"][:, 0:512],
                                 start=True, stop=False)
                nc.tensor.matmul(o, fcols(wt["win"][1], fi), st["x"][:, 512:1024],
                                 start=False, stop=False)
                nc.tensor.matmul(o, fcols(wt["whn"][0], fi), st["s"][:, 0:512],
                                 start=False, stop=False)
                nc.tensor.matmul(o, fcols(wt["whn"][1], fi), st["s"][:, 512:1024],
                                 start=False, stop=True)
            nt = n_pool.tile([128, 1024], F32, tag="n", name=f"n_{cc}")
            for fi in range(2):
                nc.scalar.activation(nt[:, fi * 512:(fi + 1) * 512],
                                     pn[:, fi * 512:(fi + 1) * 512], TANH,
                                     bias=bias_t[:, fi * 3 + 2:fi * 3 + 3])
            # out = n + z * (h - n)
            eng = nc.vector if on_dve else nc.gpsimd
            dt_ = d_pool.tile([128, 1024], F32, tag="d", name=f"d_{cc}")
            eng.tensor_sub(dt_[:, :], st["hf"][:, :], nt[:, :])
            eng.tensor_mul(dt_[:, :], st["z"][:, :], dt_[:, :])
            ot = o_pool.tile([128, 1024], F32, tag="o", name=f"o_{cc}")
            eng.tensor_add(ot[:, :], nt[:, :], dt_[:, :])
            nc.sync.dma_start(out=outL[cc], in_=ot[:, :])

        def emit_pair(c, p, hsc, xrt, rct):
            base = p * 2048
            for fi in range(2):
                pra = pr_pool.tile([128, 1024], F32, tag="pr",
                                   name=f"pr{p}f{fi}_{c}")
                # hsc pair block layout: (k-chunk, neighbor-in-pair, batch)
                nc.tensor.matmul(pra[:, 0:512], fcols(wt["whr"][0], fi),
                                 hsc[:, base:base + 512],
                                 start=True, stop=False)
                nc.tensor.matmul(pra[:, 512:1024], fcols(wt["whr"][0], fi),
                                 hsc[:, base + 512:base + 1024],
                                 start=True, stop=False)
                nc.tensor.matmul(pra[:, 0:512], fcols(wt["whr"][1], fi),
                                 hsc[:, base + 1024:base + 1536],
                                 start=False, stop=False)
                nc.tensor.matmul(pra[:, 512:1024], fcols(wt["whr"][1], fi),
                                 hsc[:, base + 1536:base + 2048],
                                 start=False, stop=False)
                nc.tensor.matmul(pra[:, 0:512], id_t[:, :],
                                 xrt[:, fi * 512:(fi + 1) * 512],
                                 start=False, stop=True)
                nc.tensor.matmul(pra[:, 512:1024], id_t[:, :],
                                 xrt[:, fi * 512:(fi + 1) * 512],
                                 start=False, stop=True)
                # r for (pair p, f chunk fi): rc cols (f, j, b) within block
                nc.scalar.activation(rct[:, base + fi * 1024:base + fi * 1024 + 1024],
                                     pra[:, :], SIG)
            # products r*hs for the whole pair block, in place in rc
            blk = slice(base, base + 2048)
            nc.vector.tensor_mul(rct[:, blk], rct[:, blk], hsc[:, blk])
            # tree level 1: j0 += j1, per f chunk (plain 2D slices)
            with nc.allow_low_precision(reason="bf16 neighbor sums"):
                for fi in range(2):
                    fb = base + fi * 1024
                    nc.vector.tensor_add(rct[:, fb:fb + 512], rct[:, fb:fb + 512],
                                         rct[:, fb + 512:fb + 1024])

        for c in range(NCH):
            # --- input DMAs (plain 2D copies, 2-4KB contiguous runs) ---
            xt = x_pool.tile([128, 1024], BF16, tag="x", name=f"x_{c}")
            nc.sync.dma_start(out=xt[:, :], in_=xL[c])
            hbt = hb_pool.tile([128, 1024], BF16, tag="hb", name=f"hb_{c}")
            nc.sync.dma_start(out=hbt[:, :], in_=hbL[c])
            hft = hf_pool.tile([128, 1024], F32, tag="hf", name=f"hf_{c}")
            nc.sync.dma_start(out=hft[:, :], in_=hfL[c])
            hsc = hs_pool.tile([128, 4 * 2048], BF16, tag="hs", name=f"hs_{c}")
            for p in range(NPAIR):
                nc.sync.dma_start(out=hsc[:, p * 2048:(p + 1) * 2048],
                                  in_=hsL[c, p])

            # --- z gate: sigmoid(Wiz@x + Whz@h + b_z) ---
            pz = pg_pool.tile([128, 1024], F32, tag="pg", name=f"pz_{c}")
            for fi in range(2):
                o = pz[:, fi * 512:(fi + 1) * 512]
                nc.tensor.matmul(o, fcols(wt["wiz"][0], fi), xt[:, 0:512],
                                 start=True, stop=False)
                nc.tensor.matmul(o, fcols(wt["wiz"][1], fi), xt[:, 512:1024],
                                 start=False, stop=False)
                nc.tensor.matmul(o, fcols(wt["whz"][0], fi), hbt[:, 0:512],
                                 start=False, stop=False)
                nc.tensor.matmul(o, fcols(wt["whz"][1], fi), hbt[:, 512:1024],
                                 start=False, stop=True)
            zt = z_pool.tile([128, 1024], F32, tag="z", name=f"z_{c}")
            for fi in range(2):
                nc.scalar.activation(zt[:, fi * 512:(fi + 1) * 512],
                                     pz[:, fi * 512:(fi + 1) * 512], SIG,
                                     bias=bias_t[:, fi * 3 + 1:fi * 3 + 2])

            # --- xr = Wir@x + b_r, bf16 ---
            px = pg_pool.tile([128, 1024], F32, tag="pg", name=f"px_{c}")
            for fi in range(2):
                o = px[:, fi * 512:(fi + 1) * 512]
                nc.tensor.matmul(o, fcols(wt["wir"][0], fi), xt[:, 0:512],
                                 start=True, stop=False)
                nc.tensor.matmul(o, fcols(wt["wir"][1], fi), xt[:, 512:1024],
                                 start=False, stop=True)
            xrt = xr_pool.tile([128, 1024], BF16, tag="xr", name=f"xr_{c}")
            for fi in range(2):
                nc.scalar.add(xrt[:, fi * 512:(fi + 1) * 512],
                              px[:, fi * 512:(fi + 1) * 512],
                              bias_t[:, fi * 3:fi * 3 + 1])

            # --- neighbor pairs; n-gate of the previous chunk in the middle
            #     so PE never waits on this chunk's product tree ---
            rct = rc_pool.tile([128, 4 * 2048], BF16, tag="rc", name=f"rc_{c}")
            emit_pair(c, 0, hsc, xrt, rct)
            emit_pair(c, 1, hsc, xrt, rct)
            if c > 0:
                emit_tail(c - 1, on_dve=False)
            emit_pair(c, 2, hsc, xrt, rct)
            emit_pair(c, 3, hsc, xrt, rct)

            # tree levels 2+3 -> s [128, (f, b)] bf16 (plain 2D slices;
            # pair p's partial for chunk fi lives at rc col p*2048 + fi*1024)
            sct = s_pool.tile([128, 1024], BF16, tag="s", name=f"s_{c}")
            with nc.allow_low_precision(reason="bf16 neighbor sums"):
                for fi in range(2):
                    o = fi * 1024
                    nc.vector.tensor_add(rct[:, o:o + 512], rct[:, o:o + 512],
                                         rct[:, 2048 + o:2048 + o + 512])
                    nc.vector.tensor_add(rct[:, 4096 + o:4096 + o + 512],
                                         rct[:, 4096 + o:4096 + o + 512],
                                         rct[:, 6144 + o:6144 + o + 512])
                    nc.vector.tensor_add(sct[:, fi * 512:(fi + 1) * 512],
                                         rct[:, o:o + 512],
                                         rct[:, 4096 + o:4096 + o + 512])

            state[c] = {"x": xt, "s": sct, "z": zt, "hf": hft}

        emit_tail(NCH - 1, on_dve=True)

    nc.compile()
    return nc


def _prep_inputs(x, h_sum, hs, Wir, bir, Whr, bhr, Wiz, biz, Whz, bhz,
                 Win, bin_, Whn, bhn):
    """Shard + pre-chunk to per-core, per-chunk feature-major HBM layouts."""
    f32 = np.float32
    x = np.asarray(x, f32)
    h = np.asarray(h_sum, f32)
    hs = np.asarray(hs, f32)

    w = {
        "wir": np.ascontiguousarray(np.asarray(Wir, f32).T.astype(BF_NP)),
        "whr": np.ascontiguousarray(np.asarray(Whr, f32).T.astype(BF_NP)),
        "wiz": np.ascontiguousarray(np.asarray(Wiz, f32).T.astype(BF_NP)),
        "whz": np.ascontiguousarray(np.asarray(Whz, f32).T.astype(BF_NP)),
        "win": np.ascontiguousarray(np.asarray(Win, f32).T.astype(BF_NP)),
        "whn": np.ascontiguousarray(np.asarray(Whn, f32).T.astype(BF_NP)),
    }
    b_r = np.asarray(bir, f32) + np.asarray(bhr, f32)
    b_z = np.asarray(biz, f32) + np.asarray(bhz, f32)
    b_n = np.asarray(bin_, f32) + np.asarray(bhn, f32)
    biasp = np.empty((128, 6), f32)
    for f in range(2):
        biasp[:, f * 3 + 0] = b_r[f * 128:(f + 1) * 128]
        biasp[:, f * 3 + 1] = b_z[f * 128:(f + 1) * 128]
        biasp[:, f * 3 + 2] = b_n[f * 128:(f + 1) * 128]
    ident = np.eye(128, dtype=f32).astype(BF_NP)

    in_maps = []
    for c in range(M):
        sl = slice(c * BL, (c + 1) * BL)
        # x/h: [BL, 256] -> [ch, b, k, p] -> [ch, p, k, b] -> [ch, 128, 1024]
        xc = x[sl].reshape(NCH, CW, 2, 128).transpose(0, 3, 2, 1)
        hc = h[sl].reshape(NCH, CW, 2, 128).transpose(0, 3, 2, 1)
        # hs: [8, BL, 256] -> [pr, j, ch, b, f, p] -> [ch, pr, p, f, j, b]
        hsc = hs[:, sl, :].reshape(NPAIR, 2, NCH, CW, 2, 128)
        hsc = hsc.transpose(2, 0, 5, 4, 1, 3)
        m = {
            "xL": xc.astype(BF_NP).reshape(NCH, 128, 1024),
            "hfL": np.ascontiguousarray(hc).reshape(NCH, 128, 1024),
            "hbL": hc.astype(BF_NP).reshape(NCH, 128, 1024),
            "hsL": hsc.astype(BF_NP).reshape(NCH, NPAIR, 128, 2048),
            "ident": ident,
            "biasp": biasp,
        }
        m.update(w)
        in_maps.append(m)
    return in_maps


def _run(inputs, trace=False, **trace_kwargs):
    global _cached
    if _cached is None:
        _cached = _build()
    nc = _cached
    in_maps = _prep_inputs(**inputs)
    res = run_bass_kernel_spmd(nc, in_maps, list(range(M)), trace=trace,
                               **trace_kwargs)
    out = np.empty((B, H), np.float32)
    for c in range(M):
        # outL [ch, p, (f b)] -> [ch, b, f, p] -> [BL, 256]
        o = res.results[c]["outL"].reshape(NCH, 128, 2, CW)
        out[c * BL:(c + 1) * BL, :] = o.transpose(0, 3, 2, 1).reshape(BL, 256)
    return out, res


def kernel(**inputs):
    return _run(inputs)[0]


# revision 14
# speedup vs baseline: 1.1414x; 1.1414x over previous
"""GRU-style GNN message-passing kernel for Trainium2 (8 NeuronCores, SPMD).

Reference computation (per node b, features 256, 8 neighbors):
    xr = x @ Wir.T + bir
    hr_n = hs_n @ Whr.T + bhr
    r_n = sigmoid(xr + hr_n)
    z = sigmoid(x @ Wiz.T + biz + h_sum @ Whz.T + bhz)
    s = sum_n r_n * hs_n
    n = tanh(x @ Win.T + bin + s @ Whn.T + bhn)
    out = (1 - z) * n + z * h_sum

Strategy: data-parallel over B=32768 across 8 cores (4096 rows each),
8 batch-chunks of 512 per core, feature-major on-chip layout
([256 features = 2 partition chunks of 128, batch free dim]).

v2 vs the first working version: engineered for PE density (the wall is
~116us of bf16 matmul columns; any PE idle gap also re-engages the HAM
clock throttle), host-side pre-chunked HBM layouts so every DMA is a
plain 2D copy with 2-4KB contiguous runs, h_sum shipped as both f32 and
bf16 (no on-chip cast), wider ACT instructions, DVE ops that all hit the
2x bf16 mode, and the n-gate of chunk c software-pipelined into the
middle of chunk c+1 so PE never waits on the DVE product tree.

Engine placement per chunk of 512 nodes:
  - PE: 68 matmuls @512 cols (z 8, xr 4, r 48 incl. per-neighbor
    identity adds of xr, n 8), PSUM-accumulated.
  - ACT: sigmoid/tanh/bias (z 2, xr 2, r 8 wide @1024, n 2).
  - DVE: r*hs pair products + neighbor add tree in bf16 2x mode; final
    combine for the last chunk (tail latency).
  - GPSIMD: final combine out = n + z*(h - n) for chunks 0..6, fp32.
"""

import sys
import numpy as np
from contextlib import ExitStack

sys.path.insert(0, "/opt/trn_rl_repo")

import ml_dtypes
import concourse.bacc as bacc
import concourse.tile as tile
from concourse import mybir
from concourse.bass_utils import run_bass_kernel_spmd

F32 = mybir.dt.float32
BF16 = mybir.dt.bfloat16
BF_NP = ml_dtypes.bfloat16

N_NEIGH, B, IN, H = 8, 32768, 256, 256
M = 8                    # cores
BL = B // M              # rows per core (4096)
NCH = 8                  # batch chunks per core
CW = BL // NCH           # chunk width (512)
NPAIR = N_NEIGH // 2     # neighbor pairs (4)

_cached = None  # compiled program, reused across kernel() calls

SIG = mybir.ActivationFunctionType.Sigmoid
TANH = mybir.ActivationFunctionType.Tanh


def _build():
    nc = bacc.Bacc("TRN2", target_bir_lowering=False, debug=False, num_devices=M)

    # xbL packs x (cols 0:1024) and h_sum-bf16 (cols 1024:2048) per chunk
    xbL = nc.dram_tensor("xbL", [NCH, 128, 2048], BF16, kind="ExternalInput").ap()
    hfL = nc.dram_tensor("hfL", [NCH, 128, 1024], F32, kind="ExternalInput").ap()
    hsL = nc.dram_tensor("hsL", [NCH, 128, 8192], BF16,
                         kind="ExternalInput").ap()
    wAP = {}
    for w in ("wir", "whr", "wiz", "whz", "win", "whn"):
        wAP[w] = nc.dram_tensor(w, [256, 256], BF16, kind="ExternalInput").ap()
    ident = nc.dram_tensor("ident", [128, 128], BF16, kind="ExternalInput").ap()
    # bias pack: col f*3+j holds feature-chunk f of (b_r, b_z, b_n)[j]
    biasp = nc.dram_tensor("biasp", [128, 6], F32, kind="ExternalInput").ap()
    outL = nc.dram_tensor("outL", [NCH, 128, 1024], F32, kind="ExternalOutput").ap()

    with tile.TileContext(nc) as tc, ExitStack() as ctx:
        cpool = ctx.enter_context(tc.tile_pool(name="const", bufs=1))
        x_pool = ctx.enter_context(tc.tile_pool(name="x", bufs=3))
        hf_pool = ctx.enter_context(tc.tile_pool(name="hf", bufs=3))
        hs_pool = ctx.enter_context(tc.tile_pool(name="hs", bufs=3))
        xr_pool = ctx.enter_context(tc.tile_pool(name="xr", bufs=2))
        z_pool = ctx.enter_context(tc.tile_pool(name="z", bufs=2))
        rc_pool = ctx.enter_context(tc.tile_pool(name="rc", bufs=2))
        pd_pool = ctx.enter_context(tc.tile_pool(name="pd", bufs=1))
        s_pool = ctx.enter_context(tc.tile_pool(name="s", bufs=2))
        n_pool = ctx.enter_context(tc.tile_pool(name="n", bufs=2))
        d_pool = ctx.enter_context(tc.tile_pool(name="d", bufs=2))
        o_pool = ctx.enter_context(tc.tile_pool(name="o", bufs=2))
        pg_pool = ctx.enter_context(tc.tile_pool(name="pg", bufs=2, space="PSUM"))
        pr_pool = ctx.enter_context(tc.tile_pool(name="pr", bufs=2, space="PSUM"))

        # --- constants ---
        wt = {}
        for w in ("wir", "whr", "wiz", "whz", "win", "whn"):
            wt[w] = []
            for k in range(2):
                t = cpool.tile([128, 256], BF16, tag=f"{w}{k}", name=f"{w}{k}")
                nc.sync.dma_start(out=t[:, :], in_=wAP[w][k * 128:(k + 1) * 128, :])
                wt[w].append(t)
        id_t = cpool.tile([128, 128], BF16, tag="ident", name="id_t")
        nc.sync.dma_start(out=id_t[:, :], in_=ident[:, :])
        bias_t = cpool.tile([128, 6], F32, tag="biasp", name="bias_t")
        nc.sync.dma_start(out=bias_t[:, :], in_=biasp[:, :])

        def fcols(t, f):
            return t[:, f * 128:(f + 1) * 128]

        state = {}  # chunk -> tiles needed by the deferred n-gate/combine

        def emit_tail(cc, on_dve):
            st = state.pop(cc)
            pn = pg_pool.tile([128, 1024], F32, tag="pg", name=f"pn_{cc}")
            for fi in range(2):
                o = pn[:, fi * 512:(fi + 1) * 512]
                nc.tensor.matmul(o, fcols(wt["win"][0], fi), st["x"][:, 0:512],
                                 start=True, stop=False)
                nc.tensor.matmul(o, fcols(wt["win"][1], fi), st["x"][:, 512:1024],
                                 start=False, stop=False)
                nc.tensor.matmul(o, fcols(wt["whn"][0], fi), st["s"][:, 0:512],
                                 start=False, stop=False)
                nc.tensor.matmul(o, fcols(wt["whn"][1], fi), st["s"][:, 512:1024],
                                 start=False, stop=True)
            nt = n_pool.tile([128, 1024], F32, tag="n", name=f"n_{cc}")
            for fi in range(2):
                nc.scalar.activation(nt[:, fi * 512:(fi + 1) * 512],
                                     pn[:, fi * 512:(fi + 1) * 512], TANH,
                                     bias=bias_t[:, fi * 3 + 2:fi * 3 + 3])
            # out = n + z * (h - n)
            eng = nc.vector if on_dve else nc.gpsimd
            dt_ = d_pool.tile([128, 1024], F32, tag="d", name=f"d_{cc}")
            eng.tensor_sub(dt_[:, :], st["hf"][:, :], nt[:, :])
            eng.tensor_mul(dt_[:, :], st["z"][:, :], dt_[:, :])
            ot = o_pool.tile([128, 1024], F32, tag="o", name=f"o_{cc}")
            eng.tensor_add(ot[:, :], nt[:, :], dt_[:, :])
            nc.sync.dma_start(out=outL[cc], in_=ot[:, :])

        def emit_pair(c, p, hsc, xrt, rct, pdt):
            base = p * 2048
            for fi in range(2):
                pra = pr_pool.tile([128, 1024], F32, tag="pr",
                                   name=f"pr{p}f{fi}_{c}")
                # hsc pair block layout: (k-chunk, neighbor-in-pair, batch)
                nc.tensor.matmul(pra[:, 0:512], fcols(wt["whr"][0], fi),
                                 hsc[:, base:base + 512],
                                 start=True, stop=False)
                nc.tensor.matmul(pra[:, 512:1024], fcols(wt["whr"][0], fi),
                                 hsc[:, base + 512:base + 1024],
                                 start=True, stop=False)
                nc.tensor.matmul(pra[:, 0:512], fcols(wt["whr"][1], fi),
                                 hsc[:, base + 1024:base + 1536],
                                 start=False, stop=False)
                nc.tensor.matmul(pra[:, 512:1024], fcols(wt["whr"][1], fi),
                                 hsc[:, base + 1536:base + 2048],
                                 start=False, stop=False)
                nc.tensor.matmul(pra[:, 0:512], id_t[:, :],
                                 xrt[:, fi * 512:(fi + 1) * 512],
                                 start=False, stop=True)
                nc.tensor.matmul(pra[:, 512:1024], id_t[:, :],
                                 xrt[:, fi * 512:(fi + 1) * 512],
                                 start=False, stop=True)
                # r for (pair p, f chunk fi): rc cols (f, j, b) within block
                nc.scalar.activation(rct[:, base + fi * 1024:base + fi * 1024 + 1024],
                                     pra[:, :], SIG)
            # products r*hs for the whole pair block -> pd (no aliasing so
            # the DVE packed bf16 mode stays eligible)
            blk = slice(base, base + 2048)
            nc.vector.tensor_mul(pdt[:, blk], rct[:, blk], hsc[:, blk])
            # tree level 1: j0 + j1 per f chunk -> rc cols [p*1024, +1024)
            # (pair p's r values there are already consumed by the product)
            with nc.allow_low_precision(reason="bf16 neighbor sums"):
                for fi in range(2):
                    nc.vector.tensor_add(
                        rct[:, p * 1024 + fi * 512:p * 1024 + fi * 512 + 512],
                        pdt[:, base + fi * 1024:base + fi * 1024 + 512],
                        pdt[:, base + fi * 1024 + 512:base + fi * 1024 + 1024])

        for c in range(NCH):
            # --- input DMAs (plain 2D copies, 4-16KB contiguous runs) ---
            xbt = x_pool.tile([128, 2048], BF16, tag="x", name=f"x_{c}")
            nc.sync.dma_start(out=xbt[:, :], in_=xbL[c])
            hft = hf_pool.tile([128, 1024], F32, tag="hf", name=f"hf_{c}")
            nc.sync.dma_start(out=hft[:, :], in_=hfL[c])
            hsc = hs_pool.tile([128, 8192], BF16, tag="hs", name=f"hs_{c}")
            nc.sync.dma_start(out=hsc[:, :], in_=hsL[c])

            # --- z gate: sigmoid(Wiz@x + Whz@h + b_z) ---
            pz = pg_pool.tile([128, 1024], F32, tag="pg", name=f"pz_{c}")
            for fi in range(2):
                o = pz[:, fi * 512:(fi + 1) * 512]
                nc.tensor.matmul(o, fcols(wt["wiz"][0], fi), xbt[:, 0:512],
                                 start=True, stop=False)
                nc.tensor.matmul(o, fcols(wt["wiz"][1], fi), xbt[:, 512:1024],
                                 start=False, stop=False)
                nc.tensor.matmul(o, fcols(wt["whz"][0], fi), xbt[:, 1024:1536],
                                 start=False, stop=False)
                nc.tensor.matmul(o, fcols(wt["whz"][1], fi), xbt[:, 1536:2048],
                                 start=False, stop=True)
            zt = z_pool.tile([128, 1024], F32, tag="z", name=f"z_{c}")
            for fi in range(2):
                nc.scalar.activation(zt[:, fi * 512:(fi + 1) * 512],
                                     pz[:, fi * 512:(fi + 1) * 512], SIG,
                                     bias=bias_t[:, fi * 3 + 1:fi * 3 + 2])

            # --- xr = Wir@x + b_r, bf16 ---
            px = pg_pool.tile([128, 1024], F32, tag="pg", name=f"px_{c}")
            for fi in range(2):
                o = px[:, fi * 512:(fi + 1) * 512]
                nc.tensor.matmul(o, fcols(wt["wir"][0], fi), xbt[:, 0:512],
                                 start=True, stop=False)
                nc.tensor.matmul(o, fcols(wt["wir"][1], fi), xbt[:, 512:1024],
                                 start=False, stop=True)
            xrt = xr_pool.tile([128, 1024], BF16, tag="xr", name=f"xr_{c}")
            for fi in range(2):
                nc.scalar.add(xrt[:, fi * 512:(fi + 1) * 512],
                              px[:, fi * 512:(fi + 1) * 512],
                              bias_t[:, fi * 3:fi * 3 + 1])

            # --- neighbor pairs; n-gate of the previous chunk in the middle
            #     so PE never waits on this chunk's product tree ---
            rct = rc_pool.tile([128, 4 * 2048], BF16, tag="rc", name=f"rc_{c}")
            pdt = pd_pool.tile([128, 4 * 2048], BF16, tag="pd", name=f"pd_{c}")
            emit_pair(c, 0, hsc, xrt, rct, pdt)
            emit_pair(c, 1, hsc, xrt, rct, pdt)
            if c > 0:
                emit_tail(c - 1, on_dve=False)
            emit_pair(c, 2, hsc, xrt, rct, pdt)
            emit_pair(c, 3, hsc, xrt, rct, pdt)

            # tree levels 2+3 -> s [128, (f, b)] bf16; pair p's L1 partial
            # lives at rc cols [p*1024, +1024). All ops alias-free.
            sct = s_pool.tile([128, 1024], BF16, tag="s", name=f"s_{c}")
            with nc.allow_low_precision(reason="bf16 neighbor sums"):
                nc.vector.tensor_add(pdt[:, 0:1024], rct[:, 0:1024],
                                     rct[:, 1024:2048])
                nc.vector.tensor_add(pdt[:, 1024:2048], rct[:, 2048:3072],
                                     rct[:, 3072:4096])
                nc.vector.tensor_add(sct[:, :], pdt[:, 0:1024],
                                     pdt[:, 1024:2048])

            state[c] = {"x": xbt, "s": sct, "z": zt, "hf": hft}

        emit_tail(NCH - 1, on_dve=True)

    nc.compile()
    return nc


def _prep_inputs(x, h_sum, hs, Wir, bir, Whr, bhr, Wiz, biz, Whz, bhz,
                 Win, bin_, Whn, bhn):
    """Shard + pre-chunk to per-core, per-chunk feature-major HBM layouts."""
    f32 = np.float32
    x = np.asarray(x, f32)
    h = np.asarray(h_sum, f32)
    hs = np.asarray(hs, f32)

    w = {
        "wir": np.ascontiguousarray(np.asarray(Wir, f32).T.astype(BF_NP)),
        "whr": np.ascontiguousarray(np.asarray(Whr, f32).T.astype(BF_NP)),
        "wiz": np.ascontiguousarray(np.asarray(Wiz, f32).T.astype(BF_NP)),
        "whz": np.ascontiguousarray(np.asarray(Whz, f32).T.astype(BF_NP)),
        "win": np.ascontiguousarray(np.asarray(Win, f32).T.astype(BF_NP)),
        "whn": np.ascontiguousarray(np.asarray(Whn, f32).T.astype(BF_NP)),
    }
    b_r = np.asarray(bir, f32) + np.asarray(bhr, f32)
    b_z = np.asarray(biz, f32) + np.asarray(bhz, f32)
    b_n = np.asarray(bin_, f32) + np.asarray(bhn, f32)
    biasp = np.empty((128, 6), f32)
    for f in range(2):
        biasp[:, f * 3 + 0] = b_r[f * 128:(f + 1) * 128]
        biasp[:, f * 3 + 1] = b_z[f * 128:(f + 1) * 128]
        biasp[:, f * 3 + 2] = b_n[f * 128:(f + 1) * 128]
    ident = np.eye(128, dtype=f32).astype(BF_NP)

    in_maps = []
    for c in range(M):
        sl = slice(c * BL, (c + 1) * BL)
        # x/h: [BL, 256] -> [ch, b, k, p] -> [ch, p, k, b] -> [ch, 128, 1024]
        xc = x[sl].reshape(NCH, CW, 2, 128).transpose(0, 3, 2, 1)
        hc = h[sl].reshape(NCH, CW, 2, 128).transpose(0, 3, 2, 1)
        xb = np.concatenate([xc.astype(BF_NP).reshape(NCH, 128, 1024),
                             hc.astype(BF_NP).reshape(NCH, 128, 1024)], axis=2)
        # hs: [8, BL, 256] -> [pr, j, ch, b, f, p] -> [ch, p, pr, f, j, b]
        hsc = hs[:, sl, :].reshape(NPAIR, 2, NCH, CW, 2, 128)
        hsc = hsc.transpose(2, 5, 0, 4, 1, 3)
        m = {
            "xbL": np.ascontiguousarray(xb),
            "hfL": np.ascontiguousarray(hc).reshape(NCH, 128, 1024),
            "hsL": hsc.astype(BF_NP).reshape(NCH, 128, 8192),
            "ident": ident,
            "biasp": biasp,
        }
        m.update(w)
        in_maps.append(m)
    return in_maps


def _run(inputs, trace=False, **trace_kwargs):
    global _cached
    if _cached is None:
        _cached = _build()
    nc = _cached
    in_maps = _prep_inputs(**inputs)
    res = run_bass_kernel_spmd(nc, in_maps, list(range(M)), trace=trace,
                               **trace_kwargs)
    out = np.empty((B, H), np.float32)
    for c in range(M):
        # outL [ch, p, (f b)] -> [ch, b, f, p] -> [BL, 256]
        o = res.results[c]["outL"].reshape(NCH, 128, 2, CW)
        out[c * BL:(c + 1) * BL, :] = o.transpose(0, 3, 2, 1).reshape(BL, 256)
    return out, res


def kernel(**inputs):
    return _run(inputs)[0]


# revision 18
# speedup vs baseline: 1.2786x; 1.1202x over previous
"""GRU-style GNN message-passing kernel for Trainium2 (8 NeuronCores, SPMD).

Reference computation (per node b, features 256, 8 neighbors):
    xr = x @ Wir.T + bir
    hr_n = hs_n @ Whr.T + bhr
    r_n = sigmoid(xr + hr_n)
    z = sigmoid(x @ Wiz.T + biz + h_sum @ Whz.T + bhz)
    s = sum_n r_n * hs_n
    n = tanh(x @ Win.T + bin + s @ Whn.T + bhn)
    out = (1 - z) * n + z * h_sum

Strategy: data-parallel over B=32768 across 8 cores (4096 rows each),
8 batch-chunks of 512 per core, feature-major on-chip layout
([256 features = 2 partition chunks of 128, batch free dim]).

v2 vs the first working version: engineered for PE density (the wall is
~116us of bf16 matmul columns; any PE idle gap also re-engages the HAM
clock throttle), host-side pre-chunked HBM layouts so every DMA is a
plain 2D copy with 2-4KB contiguous runs, h_sum shipped as both f32 and
bf16 (no on-chip cast), wider ACT instructions, DVE ops that all hit the
2x bf16 mode, and the n-gate of chunk c software-pipelined into the
middle of chunk c+1 so PE never waits on the DVE product tree.

Engine placement per chunk of 512 nodes:
  - PE: 68 matmuls @512 cols (z 8, xr 4, r 48 incl. per-neighbor
    identity adds of xr, n 8), PSUM-accumulated.
  - ACT: sigmoid/tanh/bias (z 2, xr 2, r 8 wide @1024, n 2).
  - DVE: r*hs pair products + neighbor add tree in bf16 2x mode; final
    combine for the last chunk (tail latency).
  - GPSIMD: final combine out = n + z*(h - n) for chunks 0..6, fp32.
"""

import sys
import numpy as np
from contextlib import ExitStack

sys.path.insert(0, "/opt/trn_rl_repo")

import ml_dtypes
import concourse.bacc as bacc
import concourse.tile as tile
from concourse import mybir
from concourse.bass_utils import run_bass_kernel_spmd

F32 = mybir.dt.float32
BF16 = mybir.dt.bfloat16
BF_NP = ml_dtypes.bfloat16

N_NEIGH, B, IN, H = 8, 32768, 256, 256
M = 8                    # cores
BL = B // M              # rows per core (4096)
NCH = 8                  # batch chunks per core
CW = BL // NCH           # chunk width (512)
NPAIR = N_NEIGH // 2     # neighbor pairs (4)

_cached = None  # compiled program, reused across kernel() calls

SIG = mybir.ActivationFunctionType.Sigmoid
TANH = mybir.ActivationFunctionType.Tanh


def _build():
    nc = bacc.Bacc("TRN2", target_bir_lowering=False, debug=False, num_devices=M)

    # xbL packs x (cols 0:1024) and h_sum-bf16 (cols 1024:2048) per chunk
    xbL = nc.dram_tensor("xbL", [NCH, 128, 2048], BF16, kind="ExternalInput").ap()
    hfL = nc.dram_tensor("hfL", [NCH, 128, 1024], F32, kind="ExternalInput").ap()
    hsL = nc.dram_tensor("hsL", [NCH, 128, 8192], BF16,
                         kind="ExternalInput").ap()
    wAP = {}
    for w in ("wir", "whr", "wiz", "whz", "win", "whn"):
        wAP[w] = nc.dram_tensor(w, [256, 256], BF16, kind="ExternalInput").ap()
    ident = nc.dram_tensor("ident", [128, 128], BF16, kind="ExternalInput").ap()
    # bias pack: col f*3+j holds feature-chunk f of (b_r, b_z, b_n)[j]
    biasp = nc.dram_tensor("biasp", [128, 6], F32, kind="ExternalInput").ap()
    outL = nc.dram_tensor("outL", [NCH, 128, 1024], F32, kind="ExternalOutput").ap()

    with tile.TileContext(nc) as tc, ExitStack() as ctx:
        cpool = ctx.enter_context(tc.tile_pool(name="const", bufs=1))
        x_pool = ctx.enter_context(tc.tile_pool(name="x", bufs=3))
        hf_pool = ctx.enter_context(tc.tile_pool(name="hf", bufs=3))
        hs_pool = ctx.enter_context(tc.tile_pool(name="hs", bufs=3))
        xr_pool = ctx.enter_context(tc.tile_pool(name="xr", bufs=2))
        z_pool = ctx.enter_context(tc.tile_pool(name="z", bufs=2))
        rc_pool = ctx.enter_context(tc.tile_pool(name="rc", bufs=2))
        pd_pool = ctx.enter_context(tc.tile_pool(name="pd", bufs=1))
        s_pool = ctx.enter_context(tc.tile_pool(name="s", bufs=2))
        n_pool = ctx.enter_context(tc.tile_pool(name="n", bufs=2))
        d_pool = ctx.enter_context(tc.tile_pool(name="d", bufs=2))
        o_pool = ctx.enter_context(tc.tile_pool(name="o", bufs=2))
        pp_pool = ctx.enter_context(tc.tile_pool(name="pp", bufs=2, space="PSUM"))

        # --- constants ---
        wt = {}
        for w in ("wir", "whr", "wiz", "whz", "win", "whn"):
            wt[w] = []
            for k in range(2):
                t = cpool.tile([128, 256], BF16, tag=f"{w}{k}", name=f"{w}{k}")
                nc.sync.dma_start(out=t[:, :], in_=wAP[w][k * 128:(k + 1) * 128, :])
                wt[w].append(t)
        id_t = cpool.tile([128, 128], BF16, tag="ident", name="id_t")
        nc.sync.dma_start(out=id_t[:, :], in_=ident[:, :])
        bias_t = cpool.tile([128, 6], F32, tag="biasp", name="bias_t")
        nc.sync.dma_start(out=bias_t[:, :], in_=biasp[:, :])

        def fcols(t, f):
            return t[:, f * 128:(f + 1) * 128]

        state = {}  # chunk -> tiles needed by the deferred n-gate/combine

        def emit_tail(cc, on_dve):
            st = state.pop(cc)
            pn = pp_pool.tile([128, 2048], F32, tag="pp", name=f"pn_{cc}")
            for fi in range(2):
                o = pn[:, fi * 512:(fi + 1) * 512]
                nc.tensor.matmul(o, fcols(wt["win"][0], fi), st["x"][:, 0:512],
                                 start=True, stop=False)
                nc.tensor.matmul(o, fcols(wt["win"][1], fi), st["x"][:, 512:1024],
                                 start=False, stop=False)
                nc.tensor.matmul(o, fcols(wt["whn"][0], fi), st["s"][:, 0:512],
                                 start=False, stop=False)
                nc.tensor.matmul(o, fcols(wt["whn"][1], fi), st["s"][:, 512:1024],
                                 start=False, stop=True)
            nt = n_pool.tile([128, 1024], F32, tag="n", name=f"n_{cc}")
            for fi in range(2):
                nc.scalar.activation(nt[:, fi * 512:(fi + 1) * 512],
                                     pn[:, fi * 512:(fi + 1) * 512], TANH,
                                     bias=bias_t[:, fi * 3 + 2:fi * 3 + 3])
            # out = n + z * (h - n)
            eng = nc.vector if on_dve else nc.gpsimd
            dt_ = d_pool.tile([128, 1024], F32, tag="d", name=f"d_{cc}")
            eng.tensor_sub(dt_[:, :], st["hf"][:, :], nt[:, :])
            eng.tensor_mul(dt_[:, :], st["z"][:, :], dt_[:, :])
            ot = o_pool.tile([128, 1024], F32, tag="o", name=f"o_{cc}")
            eng.tensor_add(ot[:, :], nt[:, :], dt_[:, :])
            nc.sync.dma_start(out=outL[cc], in_=ot[:, :])

        def emit_pair(c, p, hsc, xrt, rct, pdt):
            base = p * 2048
            # one PSUM tile per pair, (f, j, b) layout; four interleaved
            # 512-wide accumulation regions. whr matmuls first, the xr
            # identity adds last so xr is never waited on.
            pra = pp_pool.tile([128, 2048], F32, tag="pp", name=f"pr{p}_{c}")
            for fi in range(2):
                oj0 = pra[:, fi * 1024:fi * 1024 + 512]
                oj1 = pra[:, fi * 1024 + 512:fi * 1024 + 1024]
                nc.tensor.matmul(oj0, fcols(wt["whr"][0], fi),
                                 hsc[:, base:base + 512],
                                 start=True, stop=False)
                nc.tensor.matmul(oj1, fcols(wt["whr"][0], fi),
                                 hsc[:, base + 512:base + 1024],
                                 start=True, stop=False)
                nc.tensor.matmul(oj0, fcols(wt["whr"][1], fi),
                                 hsc[:, base + 1024:base + 1536],
                                 start=False, stop=False)
                nc.tensor.matmul(oj1, fcols(wt["whr"][1], fi),
                                 hsc[:, base + 1536:base + 2048],
                                 start=False, stop=False)
            for fi in range(2):
                nc.tensor.matmul(pra[:, fi * 1024:fi * 1024 + 512], id_t[:, :],
                                 xrt[:, fi * 512:(fi + 1) * 512],
                                 start=False, stop=True)
                nc.tensor.matmul(pra[:, fi * 1024 + 512:fi * 1024 + 1024],
                                 id_t[:, :], xrt[:, fi * 512:(fi + 1) * 512],
                                 start=False, stop=True)
            # r for pair p, both f chunks in one activation
            nc.scalar.activation(rct[:, base:base + 2048], pra[:, :], SIG)
            # products r*hs for the whole pair block -> pd (no aliasing so
            # the DVE packed bf16 mode stays eligible)
            blk = slice(base, base + 2048)
            nc.vector.tensor_mul(pdt[:, blk], rct[:, blk], hsc[:, blk])
            # tree level 1: j0 + j1 per f chunk -> rc cols [p*1024, +1024)
            # (pair p's r values there are already consumed by the product)
            with nc.allow_low_precision(reason="bf16 neighbor sums"):
                for fi in range(2):
                    nc.vector.tensor_add(
                        rct[:, p * 1024 + fi * 512:p * 1024 + fi * 512 + 512],
                        pdt[:, base + fi * 1024:base + fi * 1024 + 512],
                        pdt[:, base + fi * 1024 + 512:base + fi * 1024 + 1024])

        for c in range(NCH):
            # --- input DMAs (plain 2D copies, 4-16KB contiguous runs) ---
            xbt = x_pool.tile([128, 2048], BF16, tag="x", name=f"x_{c}")
            nc.sync.dma_start(out=xbt[:, :], in_=xbL[c])
            hft = hf_pool.tile([128, 1024], F32, tag="hf", name=f"hf_{c}")
            nc.sync.dma_start(out=hft[:, :], in_=hfL[c])
            hsc = hs_pool.tile([128, 8192], BF16, tag="hs", name=f"hs_{c}")
            nc.sync.dma_start(out=hsc[:, :], in_=hsL[c])

            # --- xr = Wir@x + b_r (cols 0:1024) then z-gate pre-act
            #     (cols 1024:2048), one shared PSUM tile; xr first so the
            #     pair identity adds never wait on it ---
            pg = pp_pool.tile([128, 2048], F32, tag="pp", name=f"pg_{c}")
            for fi in range(2):
                o = pg[:, fi * 512:(fi + 1) * 512]
                nc.tensor.matmul(o, fcols(wt["wir"][0], fi), xbt[:, 0:512],
                                 start=True, stop=False)
                nc.tensor.matmul(o, fcols(wt["wir"][1], fi), xbt[:, 512:1024],
                                 start=False, stop=True)
            for fi in range(2):
                o = pg[:, 1024 + fi * 512:1024 + (fi + 1) * 512]
                nc.tensor.matmul(o, fcols(wt["wiz"][0], fi), xbt[:, 0:512],
                                 start=True, stop=False)
                nc.tensor.matmul(o, fcols(wt["wiz"][1], fi), xbt[:, 512:1024],
                                 start=False, stop=False)
                nc.tensor.matmul(o, fcols(wt["whz"][0], fi), xbt[:, 1024:1536],
                                 start=False, stop=False)
                nc.tensor.matmul(o, fcols(wt["whz"][1], fi), xbt[:, 1536:2048],
                                 start=False, stop=True)
            xrt = xr_pool.tile([128, 1024], BF16, tag="xr", name=f"xr_{c}")
            for fi in range(2):
                nc.scalar.add(xrt[:, fi * 512:(fi + 1) * 512],
                              pg[:, fi * 512:(fi + 1) * 512],
                              bias_t[:, fi * 3:fi * 3 + 1])
            zt = z_pool.tile([128, 1024], F32, tag="z", name=f"z_{c}")
            for fi in range(2):
                nc.scalar.activation(zt[:, fi * 512:(fi + 1) * 512],
                                     pg[:, 1024 + fi * 512:1024 + (fi + 1) * 512],
                                     SIG, bias=bias_t[:, fi * 3 + 1:fi * 3 + 2])

            # --- neighbor pairs; n-gate of the previous chunk in the middle
            #     so PE never waits on this chunk's product tree ---
            rct = rc_pool.tile([128, 4 * 2048], BF16, tag="rc", name=f"rc_{c}")
            pdt = pd_pool.tile([128, 4 * 2048], BF16, tag="pd", name=f"pd_{c}")
            emit_pair(c, 0, hsc, xrt, rct, pdt)
            emit_pair(c, 1, hsc, xrt, rct, pdt)
            if c > 0:
                emit_tail(c - 1, on_dve=False)
            emit_pair(c, 2, hsc, xrt, rct, pdt)
            emit_pair(c, 3, hsc, xrt, rct, pdt)

            # tree levels 2+3 -> s [128, (f, b)] bf16; pair p's L1 partial
            # lives at rc cols [p*1024, +1024). All ops alias-free.
            sct = s_pool.tile([128, 1024], BF16, tag="s", name=f"s_{c}")
            with nc.allow_low_precision(reason="bf16 neighbor sums"):
                nc.vector.tensor_add(pdt[:, 0:1024], rct[:, 0:1024],
                                     rct[:, 1024:2048])
                nc.vector.tensor_add(pdt[:, 1024:2048], rct[:, 2048:3072],
                                     rct[:, 3072:4096])
                nc.vector.tensor_add(sct[:, :], pdt[:, 0:1024],
                                     pdt[:, 1024:2048])

            state[c] = {"x": xbt, "s": sct, "z": zt, "hf": hft}

        emit_tail(NCH - 1, on_dve=True)

    nc.compile()
    return nc


def _prep_inputs(x, h_sum, hs, Wir, bir, Whr, bhr, Wiz, biz, Whz, bhz,
                 Win, bin_, Whn, bhn):
    """Shard + pre-chunk to per-core, per-chunk feature-major HBM layouts."""
    f32 = np.float32
    x = np.asarray(x, f32)
    h = np.asarray(h_sum, f32)
    hs = np.asarray(hs, f32)

    w = {
        "wir": np.ascontiguousarray(np.asarray(Wir, f32).T.astype(BF_NP)),
        "whr": np.ascontiguousarray(np.asarray(Whr, f32).T.astype(BF_NP)),
        "wiz": np.ascontiguousarray(np.asarray(Wiz, f32).T.astype(BF_NP)),
        "whz": np.ascontiguousarray(np.asarray(Whz, f32).T.astype(BF_NP)),
        "win": np.ascontiguousarray(np.asarray(Win, f32).T.astype(BF_NP)),
        "whn": np.ascontiguousarray(np.asarray(Whn, f32).T.astype(BF_NP)),
    }
    b_r = np.asarray(bir, f32) + np.asarray(bhr, f32)
    b_z = np.asarray(biz, f32) + np.asarray(bhz, f32)
    b_n = np.asarray(bin_, f32) + np.asarray(bhn, f32)
    biasp = np.empty((128, 6), f32)
    for f in range(2):
        biasp[:, f * 3 + 0] = b_r[f * 128:(f + 1) * 128]
        biasp[:, f * 3 + 1] = b_z[f * 128:(f + 1) * 128]
        biasp[:, f * 3 + 2] = b_n[f * 128:(f + 1) * 128]
    ident = np.eye(128, dtype=f32).astype(BF_NP)

    in_maps = []
    for c in range(M):
        sl = slice(c * BL, (c + 1) * BL)
        # x/h: [BL, 256] -> [ch, b, k, p] -> [ch, p, k, b] -> [ch, 128, 1024]
        xc = x[sl].reshape(NCH, CW, 2, 128).transpose(0, 3, 2, 1)
        hc = h[sl].reshape(NCH, CW, 2, 128).transpose(0, 3, 2, 1)
        xb = np.concatenate([xc.astype(BF_NP).reshape(NCH, 128, 1024),
                             hc.astype(BF_NP).reshape(NCH, 128, 1024)], axis=2)
        # hs: [8, BL, 256] -> [pr, j, ch, b, f, p] -> [ch, p, pr, f, j, b]
        hsc = hs[:, sl, :].reshape(NPAIR, 2, NCH, CW, 2, 128)
        hsc = hsc.transpose(2, 5, 0, 4, 1, 3)
        m = {
            "xbL": np.ascontiguousarray(xb),
            "hfL": np.ascontiguousarray(hc).reshape(NCH, 128, 1024),
            "hsL": hsc.astype(BF_NP).reshape(NCH, 128, 8192),
            "ident": ident,
            "biasp": biasp,
        }
        m.update(w)
        in_maps.append(m)
    return in_maps


def _run(inputs, trace=False, **trace_kwargs):
    global _cached
    if _cached is None:
        _cached = _build()
    nc = _cached
    in_maps = _prep_inputs(**inputs)
    res = run_bass_kernel_spmd(nc, in_maps, list(range(M)), trace=trace,
                               **trace_kwargs)
    out = np.empty((B, H), np.float32)
    for c in range(M):
        # outL [ch, p, (f b)] -> [ch, b, f, p] -> [BL, 256]
        o = res.results[c]["outL"].reshape(NCH, 128, 2, CW)
        out[c * BL:(c + 1) * BL, :] = o.transpose(0, 3, 2, 1).reshape(BL, 256)
    return out, res


def kernel(**inputs):
    return _run(inputs)[0]


# revision 27
# speedup vs baseline: 1.4330x; 1.1207x over previous
"""GRU-style GNN message-passing kernel for Trainium2 (8 NeuronCores, SPMD).

Reference computation (per node b, features 256, 8 neighbors):
    xr = x @ Wir.T + bir
    hr_n = hs_n @ Whr.T + bhr
    r_n = sigmoid(xr + hr_n)
    z = sigmoid(x @ Wiz.T + biz + h_sum @ Whz.T + bhz)
    s = sum_n r_n * hs_n
    n = tanh(x @ Win.T + bin + s @ Whn.T + bhn)
    out = (1 - z) * n + z * h_sum

Strategy: data-parallel over B=32768 across 8 cores (4096 rows each),
8 batch-chunks of 512 per core, feature-major on-chip layout
([256 features = 2 partition chunks of 128, batch free dim]).

v2 vs the first working version: engineered for PE density (the wall is
~116us of bf16 matmul columns; any PE idle gap also re-engages the HAM
clock throttle), host-side pre-chunked HBM layouts so every DMA is a
plain 2D copy with 2-4KB contiguous runs, h_sum shipped as both f32 and
bf16 (no on-chip cast), wider ACT instructions, DVE ops that all hit the
2x bf16 mode, and the n-gate of chunk c software-pipelined into the
middle of chunk c+1 so PE never waits on the DVE product tree.

Engine placement per chunk of 512 nodes:
  - PE: 68 matmuls @512 cols (z 8, xr 4, r 48 incl. per-neighbor
    identity adds of xr, n 8), PSUM-accumulated.
  - ACT: sigmoid/tanh/bias (z 2, xr 2, r 8 wide @1024, n 2).
  - DVE: r*hs pair products + neighbor add tree in bf16 2x mode; final
    combine for the last chunk (tail latency).
  - GPSIMD: final combine out = n + z*(h - n) for chunks 0..6, fp32.
"""

import sys
import numpy as np
from contextlib import ExitStack

sys.path.insert(0, "/opt/trn_rl_repo")

import ml_dtypes
import concourse.bacc as bacc
import concourse.tile as tile
from concourse import mybir
from concourse.bass_utils import run_bass_kernel_spmd

F32 = mybir.dt.float32
BF16 = mybir.dt.bfloat16
BF_NP = ml_dtypes.bfloat16

N_NEIGH, B, IN, H = 8, 32768, 256, 256
M = 8                    # cores
BL = B // M              # rows per core (4096)
NCH = 8                  # batch chunks per core
CW = BL // NCH           # chunk width (512)
NPAIR = N_NEIGH // 2     # neighbor pairs (4)

_cached = None  # compiled program, reused across kernel() calls

SIG = mybir.ActivationFunctionType.Sigmoid
TANH = mybir.ActivationFunctionType.Tanh


def _build():
    nc = bacc.Bacc("TRN2", target_bir_lowering=False, debug=False, num_devices=M)

    # xbL packs x (cols 0:1024) and h_sum-bf16 (cols 1024:2048) per chunk
    xbL = nc.dram_tensor("xbL", [NCH, 128, 2048], BF16, kind="ExternalInput").ap()
    hfL = nc.dram_tensor("hfL", [NCH, 128, 1024], F32, kind="ExternalInput").ap()
    hsL = nc.dram_tensor("hsL", [NCH, 128, 8192], BF16,
                         kind="ExternalInput").ap()
    # all 12 weight chunks + the identity in one tensor: one startup DMA
    wpL = nc.dram_tensor("wpL", [128, 12 * 256 + 128], BF16,
                         kind="ExternalInput").ap()
    # bias pack: col f*3+j holds feature-chunk f of (b_r, b_z, b_n)[j]
    biasp = nc.dram_tensor("biasp", [128, 6], F32, kind="ExternalInput").ap()
    outL = nc.dram_tensor("outL", [NCH, 128, 1024], F32, kind="ExternalOutput").ap()

    W_ORDER = ("wir", "whr", "wiz", "whz", "win", "whn")

    with tile.TileContext(nc) as tc, ExitStack() as ctx:
        cpool = ctx.enter_context(tc.tile_pool(name="const", bufs=1))
        x_pool = ctx.enter_context(tc.tile_pool(name="x", bufs=3))
        hf_pool = ctx.enter_context(tc.tile_pool(name="hf", bufs=3))
        hs_pool = ctx.enter_context(tc.tile_pool(name="hs", bufs=3))
        xr_pool = ctx.enter_context(tc.tile_pool(name="xr", bufs=2))
        z_pool = ctx.enter_context(tc.tile_pool(name="z", bufs=2))
        rc_pool = ctx.enter_context(tc.tile_pool(name="rc", bufs=2))
        pd_pool = ctx.enter_context(tc.tile_pool(name="pd", bufs=2))
        s_pool = ctx.enter_context(tc.tile_pool(name="s", bufs=2))
        n_pool = ctx.enter_context(tc.tile_pool(name="n", bufs=2))
        d_pool = ctx.enter_context(tc.tile_pool(name="d", bufs=2))
        o_pool = ctx.enter_context(tc.tile_pool(name="o", bufs=2))
        pp_pool = ctx.enter_context(tc.tile_pool(name="pp", bufs=2, space="PSUM"))

        # --- constants: one big weight DMA + the bias columns ---
        wp_t = cpool.tile([128, 12 * 256 + 128], BF16, tag="wp", name="wp_t")
        nc.sync.dma_start(out=wp_t[:, :], in_=wpL[:, :])
        bias_t = cpool.tile([128, 6], F32, tag="biasp", name="bias_t")
        nc.sync.dma_start(out=bias_t[:, :], in_=biasp[:, :])

        wt = {w: [wp_t[:, (2 * i + k) * 256:(2 * i + k + 1) * 256]
                  for k in range(2)]
              for i, w in enumerate(W_ORDER)}
        id_t = wp_t[:, 12 * 256:12 * 256 + 128]

        def fcols(t, f):
            return t[:, f * 128:(f + 1) * 128]

        state = {}  # chunk -> tiles needed by the deferred n-gate/combine

        def emit_tail(cc, fine=False):
            # n-gate + combine (on DVE) + store for chunk cc.  fine=True
            # pipelines per f-half to shorten the end-of-kernel chain.
            st = state.pop(cc)
            pn = pp_pool.tile([128, 2048], F32, tag="pp", name=f"pn_{cc}")
            nt = n_pool.tile([128, 1024], F32, tag="n", name=f"n_{cc}")
            dt_ = d_pool.tile([128, 1024], F32, tag="d", name=f"d_{cc}")
            ot = o_pool.tile([128, 1024], F32, tag="o", name=f"o_{cc}")

            def mm_f(fi):
                o = pn[:, fi * 512:(fi + 1) * 512]
                nc.tensor.matmul(o, fcols(wt["win"][0], fi), st["x"][:, 0:512],
                                 start=True, stop=False)
                nc.tensor.matmul(o, fcols(wt["win"][1], fi), st["x"][:, 512:1024],
                                 start=False, stop=False)
                nc.tensor.matmul(o, fcols(wt["whn"][0], fi), st["s"][:, 0:512],
                                 start=False, stop=False)
                nc.tensor.matmul(o, fcols(wt["whn"][1], fi), st["s"][:, 512:1024],
                                 start=False, stop=True)

            def act_f(fi):
                nc.scalar.activation(nt[:, fi * 512:(fi + 1) * 512],
                                     pn[:, fi * 512:(fi + 1) * 512], TANH,
                                     bias=bias_t[:, fi * 3 + 2:fi * 3 + 3])

            def comb_f(fi, width=512):
                s_ = slice(fi * 512, fi * 512 + width)
                nc.vector.tensor_sub(dt_[:, s_], st["hf"][:, s_], nt[:, s_])
                nc.vector.tensor_mul(dt_[:, s_], st["z"][:, s_], dt_[:, s_])
                nc.vector.tensor_add(ot[:, s_], nt[:, s_], dt_[:, s_])
                nc.sync.dma_start(out=outL[cc][:, s_], in_=ot[:, s_])

            if fine:
                for fi in range(2):
                    mm_f(fi)
                    act_f(fi)
                    comb_f(fi)
            else:
                mm_f(0)
                mm_f(1)
                act_f(0)
                act_f(1)
                comb_f(0, width=1024)

        def emit_pair(c, p, hsc, xrt, rct, pdt):
            base = p * 2048
            # one PSUM tile per pair, (f, j, b) layout; four interleaved
            # 512-wide accumulation regions. whr matmuls first, the xr
            # identity adds last so xr is never waited on.
            pra = pp_pool.tile([128, 2048], F32, tag="pp", name=f"pr{p}_{c}")
            for fi in range(2):
                oj0 = pra[:, fi * 1024:fi * 1024 + 512]
                oj1 = pra[:, fi * 1024 + 512:fi * 1024 + 1024]
                nc.tensor.matmul(oj0, fcols(wt["whr"][0], fi),
                                 hsc[:, base:base + 512],
                                 start=True, stop=False)
                nc.tensor.matmul(oj1, fcols(wt["whr"][0], fi),
                                 hsc[:, base + 512:base + 1024],
                                 start=True, stop=False)
                nc.tensor.matmul(oj0, fcols(wt["whr"][1], fi),
                                 hsc[:, base + 1024:base + 1536],
                                 start=False, stop=False)
                nc.tensor.matmul(oj1, fcols(wt["whr"][1], fi),
                                 hsc[:, base + 1536:base + 2048],
                                 start=False, stop=False)
            for fi in range(2):
                nc.tensor.matmul(pra[:, fi * 1024:fi * 1024 + 512], id_t[:, :],
                                 xrt[:, fi * 512:(fi + 1) * 512],
                                 start=False, stop=True)
                nc.tensor.matmul(pra[:, fi * 1024 + 512:fi * 1024 + 1024],
                                 id_t[:, :], xrt[:, fi * 512:(fi + 1) * 512],
                                 start=False, stop=True)
            # r for pair p, both f chunks in one activation
            nc.scalar.activation(rct[:, base:base + 2048], pra[:, :], SIG)
            # products r*hs for the whole pair block -> pd (no aliasing so
            # the DVE packed bf16 mode stays eligible)
            blk = slice(base, base + 2048)
            nc.vector.tensor_mul(pdt[:, blk], rct[:, blk], hsc[:, blk])
            # tree level 1: j0 + j1 per f chunk -> rc cols [p*1024, +1024)
            # (pair p's r values there are already consumed by the product)
            with nc.allow_low_precision(reason="bf16 neighbor sums"):
                for fi in range(2):
                    nc.vector.tensor_add(
                        rct[:, p * 1024 + fi * 512:p * 1024 + fi * 512 + 512],
                        pdt[:, base + fi * 1024:base + fi * 1024 + 512],
                        pdt[:, base + fi * 1024 + 512:base + fi * 1024 + 1024])

        pend = {}  # chunk -> (rct, pdt) awaiting tree levels 2+3

        def emit_l23(cc):
            rct, pdt = pend.pop(cc)
            sct = s_pool.tile([128, 1024], BF16, tag="s", name=f"s_{cc}")
            with nc.allow_low_precision(reason="bf16 neighbor sums"):
                nc.vector.tensor_add(pdt[:, 0:1024], rct[:, 0:1024],
                                     rct[:, 1024:2048])
                nc.vector.tensor_add(pdt[:, 1024:2048], rct[:, 2048:3072],
                                     rct[:, 3072:4096])
                nc.vector.tensor_add(sct[:, :], pdt[:, 0:1024],
                                     pdt[:, 1024:2048])
            state[cc]["s"] = sct

        for c in range(NCH):
            # --- input DMAs (plain 2D copies, 4-16KB contiguous runs);
            #     chunk 0's hs comes in per-pair so pair0 lands early ---
            xbt = x_pool.tile([128, 2048], BF16, tag="x", name=f"x_{c}")
            nc.sync.dma_start(out=xbt[:, :], in_=xbL[c])
            hsc = hs_pool.tile([128, 8192], BF16, tag="hs", name=f"hs_{c}")
            if c == 0:
                for p in range(NPAIR):
                    nc.sync.dma_start(out=hsc[:, p * 2048:(p + 1) * 2048],
                                      in_=hsL[c][:, p * 2048:(p + 1) * 2048])
            else:
                nc.sync.dma_start(out=hsc[:, :], in_=hsL[c])
            hft = hf_pool.tile([128, 1024], F32, tag="hf", name=f"hf_{c}")
            nc.sync.dma_start(out=hft[:, :], in_=hfL[c])

            # tree tail of the previous chunk opens the DVE stream here,
            # filling what would otherwise be a DVE idle (re-throttle) gap
            if c > 0:
                emit_l23(c - 1)

            # --- xr = Wir@x + b_r (cols 0:1024) then z-gate pre-act
            #     (cols 1024:2048), one shared PSUM tile; xr first so the
            #     pair identity adds never wait on it ---
            pg = pp_pool.tile([128, 2048], F32, tag="pp", name=f"pg_{c}")
            for fi in range(2):
                o = pg[:, fi * 512:(fi + 1) * 512]
                nc.tensor.matmul(o, fcols(wt["wir"][0], fi), xbt[:, 0:512],
                                 start=True, stop=False)
                nc.tensor.matmul(o, fcols(wt["wir"][1], fi), xbt[:, 512:1024],
                                 start=False, stop=True)
            for fi in range(2):
                o = pg[:, 1024 + fi * 512:1024 + (fi + 1) * 512]
                nc.tensor.matmul(o, fcols(wt["wiz"][0], fi), xbt[:, 0:512],
                                 start=True, stop=False)
                nc.tensor.matmul(o, fcols(wt["wiz"][1], fi), xbt[:, 512:1024],
                                 start=False, stop=False)
                nc.tensor.matmul(o, fcols(wt["whz"][0], fi), xbt[:, 1024:1536],
                                 start=False, stop=False)
                nc.tensor.matmul(o, fcols(wt["whz"][1], fi), xbt[:, 1536:2048],
                                 start=False, stop=True)
            xrt = xr_pool.tile([128, 1024], BF16, tag="xr", name=f"xr_{c}")
            for fi in range(2):
                nc.scalar.add(xrt[:, fi * 512:(fi + 1) * 512],
                              pg[:, fi * 512:(fi + 1) * 512],
                              bias_t[:, fi * 3:fi * 3 + 1])
            zt = z_pool.tile([128, 1024], F32, tag="z", name=f"z_{c}")
            for fi in range(2):
                nc.scalar.activation(zt[:, fi * 512:(fi + 1) * 512],
                                     pg[:, 1024 + fi * 512:1024 + (fi + 1) * 512],
                                     SIG, bias=bias_t[:, fi * 3 + 1:fi * 3 + 2])

            # --- neighbor pairs; n-gate of the previous chunk in the middle
            #     so PE never waits on this chunk's product tree ---
            rct = rc_pool.tile([128, 4 * 2048], BF16, tag="rc", name=f"rc_{c}")
            pdt = pd_pool.tile([128, 4 * 2048], BF16, tag="pd", name=f"pd_{c}")
            state[c] = {"x": xbt, "z": zt, "hf": hft}
            emit_pair(c, 0, hsc, xrt, rct, pdt)
            emit_pair(c, 1, hsc, xrt, rct, pdt)
            if c > 0:
                emit_tail(c - 1)
            emit_pair(c, 2, hsc, xrt, rct, pdt)
            emit_pair(c, 3, hsc, xrt, rct, pdt)
            pend[c] = (rct, pdt)

        emit_l23(NCH - 1)
        emit_tail(NCH - 1, fine=True)

    nc.compile()
    return nc


def _prep_inputs(x, h_sum, hs, Wir, bir, Whr, bhr, Wiz, biz, Whz, bhz,
                 Win, bin_, Whn, bhn):
    """Shard + pre-chunk to per-core, per-chunk feature-major HBM layouts."""
    f32 = np.float32
    x = np.asarray(x, f32)
    h = np.asarray(h_sum, f32)
    hs = np.asarray(hs, f32)

    # packed weights: wpL[p, (2i+k)*256 + f*128 + m] = W_i[f*128+m, k*128+p];
    # trailing 128 cols hold the identity
    wpack = np.zeros((128, 12 * 256 + 128), f32)
    for i, W in enumerate((Wir, Whr, Wiz, Whz, Win, Whn)):
        WT = np.asarray(W, f32).T  # [in, out]
        for k in range(2):
            wpack[:, (2 * i + k) * 256:(2 * i + k + 1) * 256] = \
                WT[k * 128:(k + 1) * 128, :]
    wpack[:, 12 * 256:] = np.eye(128, dtype=f32)
    wpack_bf = np.ascontiguousarray(wpack.astype(BF_NP))
    b_r = np.asarray(bir, f32) + np.asarray(bhr, f32)
    b_z = np.asarray(biz, f32) + np.asarray(bhz, f32)
    b_n = np.asarray(bin_, f32) + np.asarray(bhn, f32)
    biasp = np.empty((128, 6), f32)
    for f in range(2):
        biasp[:, f * 3 + 0] = b_r[f * 128:(f + 1) * 128]
        biasp[:, f * 3 + 1] = b_z[f * 128:(f + 1) * 128]
        biasp[:, f * 3 + 2] = b_n[f * 128:(f + 1) * 128]

    in_maps = []
    for c in range(M):
        sl = slice(c * BL, (c + 1) * BL)
        # x/h: [BL, 256] -> [ch, b, k, p] -> [ch, p, k, b] -> [ch, 128, 1024]
        xc = x[sl].reshape(NCH, CW, 2, 128).transpose(0, 3, 2, 1)
        hc = h[sl].reshape(NCH, CW, 2, 128).transpose(0, 3, 2, 1)
        xb = np.concatenate([xc.astype(BF_NP).reshape(NCH, 128, 1024),
                             hc.astype(BF_NP).reshape(NCH, 128, 1024)], axis=2)
        # hs: [8, BL, 256] -> [pr, j, ch, b, f, p] -> [ch, p, pr, f, j, b]
        hsc = hs[:, sl, :].reshape(NPAIR, 2, NCH, CW, 2, 128)
        hsc = hsc.transpose(2, 5, 0, 4, 1, 3)
        m = {
            "xbL": np.ascontiguousarray(xb),
            "hfL": np.ascontiguousarray(hc).reshape(NCH, 128, 1024),
            "hsL": hsc.astype(BF_NP).reshape(NCH, 128, 8192),
            "wpL": wpack_bf,
            "biasp": biasp,
        }
        in_maps.append(m)
    return in_maps


def _run(inputs, trace=False, **trace_kwargs):
    global _cached
    if _cached is None:
        _cached = _build()
    nc = _cached
    in_maps = _prep_inputs(**inputs)
    res = run_bass_kernel_spmd(nc, in_maps, list(range(M)), trace=trace,
                               **trace_kwargs)
    out = np.empty((B, H), np.float32)
    for c in range(M):
        # outL [ch, p, (f b)] -> [ch, b, f, p] -> [BL, 256]
        o = res.results[c]["outL"].reshape(NCH, 128, 2, CW)
        out[c * BL:(c + 1) * BL, :] = o.transpose(0, 3, 2, 1).reshape(BL, 256)
    return out, res


def kernel(**inputs):
    return _run(inputs)[0]
